# revision 17
# baseline (speedup 1.0000x reference)
"""Trainium2 Bass kernel for a LoRA-MoE layer (gate top-2 softmax routing +
dense base linear + per-expert low-rank adapters), SPMD across 8 NeuronCores.

Math (per token t):
    logits = x @ gate_w.T                      # [E]
    top-2 softmax over logits -> dense w[E] (0 for non-selected)
    out = x @ base_w.T + base_b
        + SCALING * sum_e w[e] * (x @ lora_A[e].T) @ lora_B[e].T

Key identities:
  * w folded into rank-space activations: lora_out = (low * w_rep) @ B_all.T
    with low = x @ A_all.T (A_all: [E*R, D]) -> whole MoE-LoRA is two dense
    matmuls + tiny gating vector math.
  * top-2 softmax via sigmoid: w_e = [l_e >= m2] * sigmoid(2*l_e - m1 - m2)
    (for the top-1 expert this is sigmoid(l1-l2), for top-2 sigmoid(l2-l1)).

Sharding: 8-way data parallel over tokens (T=512 tokens per core), base W
replicated and streamed.  This halves the x-load + phase-A serial head vs a
token x out-feature split; W streaming needs only ~150 GB/s per core.

Performance structure:
  * all matmul operands bf16 (host cast, free) -> PE rate unchanged, HBM
    bytes halved.
  * DMA order: adapters + x chunks first on both rings, W strictly behind x.
  * ~16 dummy matmuls at t~4us keep the PE HAM clock-gate warm before the
    first x chunk lands.
  * single shared 8-slot PSUM pool; out-tile k-loops run back-to-back while
    the gating vector chain (DVE/ACT/GPSIMD) hides behind them; each
    out-tile's B-adapter "stop" matmul is deferred two groups.
"""

import numpy as np
import ml_dtypes

import concourse.bass as bass
import concourse.bass_isa as bass_isa
import concourse.mybir as mybir
import concourse.tile as tile
from concourse import bacc
from concourse.bass_utils import run_bass_kernel_spmd

F32 = mybir.dt.float32
BF16 = mybir.dt.bfloat16
NPBF16 = ml_dtypes.bfloat16

# Problem constants
B, S, D, O = 2, 2048, 4096, 4096
E, R = 8, 16
ER = E * R  # 128
SCALING = 32.0 / 16.0

# Sharding: 8 token groups, W replicated
N_CORES = 8
TG = 8
T = (B * S) // TG       # 512 tokens per core
KT = D // 128           # 32 contraction tiles
OTN = O // 128          # 32 out tiles per core
XC = 8                  # x DMA chunks
KPC = KT // XC          # 4 k-tiles per chunk
NWARM = 10             # PE warm-up matmuls


def build_body(nc, tc, tensors):
    xT, wT, aT, gT, bT, bias2, Rm, out = tensors
    OP = mybir.AluOpType
    ACT = mybir.ActivationFunctionType

    with (
        tc.tile_pool(name="xp", bufs=XC) as xp,
        tc.tile_pool(name="wp", bufs=8) as wp,
        tc.tile_pool(name="cst", bufs=1) as cst,
        tc.tile_pool(name="gw", bufs=1) as gw,
        tc.tile_pool(name="outp", bufs=4) as outp,
        tc.tile_pool(name="ps", bufs=8, space="PSUM") as ps,
    ):
        # ---- DMA program.  The gate/lora-A/x slices for each group of 4
        #      k-tiles are fused host-side into one contiguous per-partition
        #      payload (~5KB lines, no small packets); chunks alternate
        #      across the sync and scalar rings.  W queues strictly behind
        #      the chunks on the sync ring; bias/Rm/bT slot in between the
        #      first W tiles (needed only from ~40us). ----
        PK = E + ER + T          # one k-tile's payload (g | a | x)
        CW = KPC * PK
        w_tiles = [wp.tile([128, KT * 128], BF16, tag="w", name=f"w{ot}")
                   for ot in range(OTN)]

        # even chunks + the first four W tiles interleaved on the (faster)
        # sync ring, odd chunks on the scalar ring; W0..W3 feed the
        # staggered prefix k-loops that fill the PE during the x stream.
        x_tiles = []
        for c in range(XC):
            eng = nc.sync if c % 2 == 0 else nc.scalar
            xc_t = xp.tile([128, CW], BF16, tag="x", name=f"x{c}")
            if c == 0:
                # split the first chunk so phase A starts ~1.5us earlier
                eng.dma_start(out=xc_t[:, :2 * PK], in_=xT[:, c, :2 * PK])
                eng.dma_start(out=xc_t[:, 2 * PK:], in_=xT[:, c, 2 * PK:])
            else:
                eng.dma_start(out=xc_t[:], in_=xT[:, c, :])
            x_tiles.append(xc_t)
            if c % 2 == 0:
                j = c // 2
                nc.sync.dma_start(out=w_tiles[j][:], in_=wT[:, j, :])

        # the scheduler would otherwise hoist these ahead of the x chunks
        # (stealing completion sems from them); hold them past the x window
        with tc.tile_wait_until(0.022):
            bias_sb = cst.tile([128, OTN], F32)
            nc.sync.dma_start(out=bias_sb[:], in_=bias2[:])
            Rm_sb = cst.tile([E, ER], BF16)
            nc.sync.dma_start(out=Rm_sb[:], in_=Rm[:])
            bT_sb = cst.tile([ER, O], BF16)
            nc.sync.dma_start(out=bT_sb[:], in_=bT[:])

        with tc.tile_wait_until(0.024):
            for ot in range(4, OTN):
                nc.sync.dma_start(out=w_tiles[ot][:], in_=wT[:, ot, :])

        def gs(k):
            """gate_w.T slice [128, E] for k-tile k."""
            o = (k % KPC) * PK
            return x_tiles[k // KPC][:, o:o + E]

        def as_(k):
            """lora_A.T slice [128, ER] for k-tile k."""
            o = (k % KPC) * PK + E
            return x_tiles[k // KPC][:, o:o + ER]

        def xs(k):
            """x.T slice [128, T] for k-tile k."""
            o = (k % KPC) * PK + E + ER
            return x_tiles[k // KPC][:, o:o + T]

        # ---- PE warm-up: dummy matmuls on zeros so the HAM clock gate is
        #      already at 8/8 when the first x chunk lands.  They write the
        #      gate PSUM bank; the real k=0 matmul's start=True wipes them. ----
        warm = cst.tile([128, T], BF16)
        nc.vector.memset(warm[:], 0.0)

        gate_ps = ps.tile([E, T], F32, tag="ps", name="gateps")
        low_ps = ps.tile([ER, T], F32, tag="ps", name="lowps")
        for i in range(NWARM):
            nc.tensor.matmul(gate_ps[:], lhsT=warm[:, :E], rhs=warm[:],
                             start=True, stop=True, skip_group_check=True)

        # ---- phase B group helper (one chunk of an out-tile's k-loop) ----
        pbs = [None] * OTN

        def w_chunk(ot, cc):
            for k in range(cc * KPC, (cc + 1) * KPC):
                nc.tensor.matmul(pbs[ot][:],
                                 lhsT=w_tiles[ot][:, k * 128:(k + 1) * 128],
                                 rhs=xs(k), start=(k == 0), stop=False)

        # ---- phase A (gate/low k-loops) streamed against chunk arrival,
        #      with W0..W3's k-loops staggered in to fill the PE while the
        #      remaining x chunks are still in flight ----
        for j in range(4):
            pbs[j] = ps.tile([128, T], F32, tag="ps", name=f"pb{j}")
        for c in range(XC):
            for k in range(c * KPC, (c + 1) * KPC):
                nc.tensor.matmul(gate_ps[:], lhsT=gs(k),
                                 rhs=xs(k), start=(k == 0), stop=(k == KT - 1),
                                 skip_group_check=(k == 0))
                nc.tensor.matmul(low_ps[:], lhsT=as_(k),
                                 rhs=xs(k), start=(k == 0), stop=(k == KT - 1))
            for j in range(4):
                cc = c - 2 - j
                if cc >= 0:
                    w_chunk(j, cc)
        for j in range(4):
            for cc in range(XC - 2 - j, XC):
                w_chunk(j, cc)

        # ---- gating math in [E, t] layout (DVE/ACT/GPSIMD; overlaps the
        #      first base-W matmul groups on the PE) ----
        # w_e = [l_e >= m2] * sigmoid(2*l_e - m1 - m2) * SCALING
        lowT_sb = cst.tile([ER, T], BF16, tag="lowT")
        g_sb = gw.tile([E, T], F32, tag="g")
        nc.scalar.copy(g_sb[:], gate_ps[:])
        m1 = gw.tile([E, T], F32, tag="m1")
        nc.gpsimd.partition_all_reduce(m1[:], g_sb[:], channels=E,
                                       reduce_op=bass_isa.ReduceOp.max)
        eq = gw.tile([E, T], F32, tag="eq")
        nc.vector.tensor_tensor(eq[:], g_sb[:], m1[:], op=OP.is_equal)
        gm = gw.tile([E, T], F32, tag="gm")
        nc.vector.scalar_tensor_tensor(gm[:], in0=eq[:], scalar=-1e30,
                                       in1=g_sb[:], op0=OP.mult, op1=OP.add)
        m2 = gw.tile([E, T], F32, tag="m2")
        nc.gpsimd.partition_all_reduce(m2[:], gm[:], channels=E,
                                       reduce_op=bass_isa.ReduceOp.max)
        t1 = gw.tile([E, T], F32, tag="t1")
        nc.vector.tensor_tensor(t1[:], m1[:], m2[:], op=OP.add)
        s = gw.tile([E, T], F32, tag="s")
        nc.vector.scalar_tensor_tensor(s[:], in0=g_sb[:], scalar=2.0,
                                       in1=t1[:], op0=OP.mult, op1=OP.subtract)
        sig = gw.tile([E, T], F32, tag="sig")
        nc.scalar.activation(sig[:], s[:], ACT.Sigmoid)
        mask = gw.tile([E, T], F32, tag="mask")
        nc.vector.tensor_tensor(mask[:], g_sb[:], m2[:], op=OP.is_ge)
        wsc = gw.tile([E, T], BF16, tag="wsc")
        nc.vector.scalar_tensor_tensor(wsc[:], in0=sig[:], scalar=SCALING,
                                       in1=mask[:], op0=OP.mult, op1=OP.mult)

        # ---- phase B: out.T[ot] = sum_k W[ot,k]^T @ x.T (+ B^T @ low_w.T) ----
        def stop_group(ot):
            nc.tensor.matmul(pbs[ot][:], lhsT=bT_sb[:, ot * 128:(ot + 1) * 128],
                             rhs=lowT_sb[:], start=False, stop=True)
            o_sb = outp.tile([128, T], BF16, tag="o", name=f"o{ot}")
            nc.vector.tensor_scalar(o_sb[:], pbs[ot][:],
                                    scalar1=bias_sb[:, ot:ot + 1],
                                    scalar2=None, op0=OP.add)
            # last few outputs go out on the otherwise-idle ACT ring so the
            # final drain doesn't serialize behind one DMA queue
            eng = nc.scalar if ot >= OTN - 3 else nc.gpsimd
            eng.dma_start(out=out[:, ot, :], in_=o_sb[:])

        for ot in range(4, OTN):
            pbs[ot] = ps.tile([128, T], F32, tag="ps", name=f"pb{ot}")
            if ot == OTN - 1:
                # close out the previous group before the final W k-loop so
                # only one stop+store trails the last matmul
                stop_group(OTN - 2)
            for cc in range(XC):
                w_chunk(ot, cc)
            if ot == 4:
                # replicate each expert weight over its 16 ranks via a tiny
                # matmul, then fold into the rank-space activations.  Emitted
                # after ot4's W MMs so the PE never waits on the gating chain.
                wrep_ps = ps.tile([ER, T], F32, tag="ps", name="wrep")
                nc.tensor.matmul(wrep_ps[:], lhsT=Rm_sb[:], rhs=wsc[:],
                                 start=True, stop=True)
                wrep_sb = gw.tile([ER, T], F32, tag="wrepsb")
                nc.scalar.copy(wrep_sb[:], wrep_ps[:])
                # low_w.T = low.T * w_rep (DVE: one PSUM operand only)
                nc.vector.tensor_tensor(lowT_sb[:], low_ps[:], wrep_sb[:],
                                        op=OP.mult)
            if ot == 5:
                for j in range(5):
                    stop_group(j)
            if ot >= 6 and ot < OTN - 1:
                stop_group(ot - 1)
        stop_group(OTN - 1)


def build_module(debug=False):
    nc = bacc.Bacc("TRN2", target_bir_lowering=False, debug=debug)
    CW = KPC * (E + ER + T)
    xT = nc.dram_tensor("xT", [128, XC, CW], BF16, kind="ExternalInput")
    wT = nc.dram_tensor("wT", [128, OTN, KT * 128], BF16, kind="ExternalInput")
    aT = None
    gT = None
    bT = nc.dram_tensor("bT", [ER, O], BF16, kind="ExternalInput")
    bias2 = nc.dram_tensor("bias2", [128, OTN], F32, kind="ExternalInput")
    Rm = nc.dram_tensor("Rm", [E, ER], BF16, kind="ExternalInput")
    out = nc.dram_tensor("out", [128, OTN, T], BF16, kind="ExternalOutput")
    with tile.TileContext(nc) as tc:
        build_body(nc, tc, (xT, wT, aT, gT, bT, bias2, Rm, out))
    nc.compile()
    return nc


def shard_inputs(x, gate_w, base_w, base_b, lora_A, lora_B):
    """FULL inputs -> list of 8 per-core input maps (host-side, free)."""
    x = np.asarray(x, dtype=np.float32)
    gate_w = np.asarray(gate_w, dtype=np.float32)
    base_w = np.asarray(base_w, dtype=np.float32)
    base_b = np.asarray(base_b, dtype=np.float32)
    lora_A = np.asarray(lora_A, dtype=np.float32)
    lora_B = np.asarray(lora_B, dtype=np.float32)

    xf = x.reshape(B * S, D)
    # replicated tensors; gate/lora-A slices fused into each x chunk payload
    gp = gate_w.T.reshape(XC, KPC, 128, E).transpose(2, 0, 1, 3)  # [128,XC,KPC,E]
    A_flat = lora_A.reshape(ER, D)
    ap = A_flat.T.reshape(XC, KPC, 128, ER).transpose(2, 0, 1, 3)
    B_flat = lora_B.transpose(0, 2, 1).reshape(ER, O)   # [er, o]
    bT = np.ascontiguousarray(B_flat).astype(NPBF16)
    Rm = np.repeat(np.eye(E, dtype=np.float32), R, axis=1).astype(NPBF16)
    wT = np.ascontiguousarray(
        base_w.reshape(OTN, 128, KT, 128).transpose(3, 0, 2, 1)
        .reshape(128, OTN, KT * 128)).astype(NPBF16)
    bias2 = np.ascontiguousarray(base_b.reshape(OTN, 128).T)
    in_maps = []
    for c in range(N_CORES):
        x_c = xf[c * T:(c + 1) * T]                         # [T, D]
        xp_ = x_c.T.reshape(XC, KPC, 128, T).transpose(2, 0, 1, 3)
        # per-k payload blocks [g_k | a_k | x_k], flattened per chunk
        xT = np.ascontiguousarray(np.concatenate(
            [gp, ap, xp_], axis=3).reshape(128, XC, KPC * (E + ER + T))
            ).astype(NPBF16)
        in_maps.append({"xT": xT, "wT": wT,
                        "bT": bT, "bias2": bias2, "Rm": Rm})
    return in_maps


def gather_outputs(results):
    """list of 8 per-core result maps -> FULL output [B, S, O]."""
    full = np.empty((B * S, O), dtype=np.float32)
    for c in range(N_CORES):
        oc = np.asarray(results[c]["out"], dtype=np.float32)  # [128, OTN, T]
        full[c * T:(c + 1) * T, :] = oc.transpose(2, 1, 0).reshape(T, O)
    return full.reshape(B, S, O)


_NC_CACHE = {}


def _get_module():
    if "nc" not in _NC_CACHE:
        _NC_CACHE["nc"] = build_module()
    return _NC_CACHE["nc"]


def run_sharded(in_maps, **run_kwargs):
    nc = _get_module()
    return run_bass_kernel_spmd(nc, in_maps, list(range(N_CORES)), **run_kwargs)


def kernel(x, gate_w, base_w, base_b, lora_A, lora_B):
    in_maps = shard_inputs(x, gate_w, base_w, base_b, lora_A, lora_B)
    res = run_sharded(in_maps)
    return gather_outputs(res.results)


# revision 18
# speedup vs baseline: 1.0184x; 1.0184x over previous
"""Trainium2 Bass kernel for a LoRA-MoE layer (gate top-2 softmax routing +
dense base linear + per-expert low-rank adapters), SPMD across 8 NeuronCores.

Math (per token t):
    logits = x @ gate_w.T                      # [E]
    top-2 softmax over logits -> dense w[E] (0 for non-selected)
    out = x @ base_w.T + base_b
        + SCALING * sum_e w[e] * (x @ lora_A[e].T) @ lora_B[e].T

Key identities:
  * w folded into rank-space activations: lora_out = (low * w_rep) @ B_all.T
    with low = x @ A_all.T (A_all: [E*R, D]) -> whole MoE-LoRA is two dense
    matmuls + tiny gating vector math.
  * top-2 softmax via sigmoid: w_e = [l_e >= m2] * sigmoid(2*l_e - m1 - m2)
    (for the top-1 expert this is sigmoid(l1-l2), for top-2 sigmoid(l2-l1)).

Sharding: 8-way data parallel over tokens (T=512 tokens per core), base W
replicated and streamed.  This halves the x-load + phase-A serial head vs a
token x out-feature split; W streaming needs only ~150 GB/s per core.

Performance structure:
  * all matmul operands bf16 (host cast, free) -> PE rate unchanged, HBM
    bytes halved.
  * DMA order: adapters + x chunks first on both rings, W strictly behind x.
  * ~16 dummy matmuls at t~4us keep the PE HAM clock-gate warm before the
    first x chunk lands.
  * single shared 8-slot PSUM pool; out-tile k-loops run back-to-back while
    the gating vector chain (DVE/ACT/GPSIMD) hides behind them; each
    out-tile's B-adapter "stop" matmul is deferred two groups.
"""

import numpy as np
import ml_dtypes

import concourse.bass as bass
import concourse.bass_isa as bass_isa
import concourse.mybir as mybir
import concourse.tile as tile
from concourse import bacc
from concourse.bass_utils import run_bass_kernel_spmd

F32 = mybir.dt.float32
BF16 = mybir.dt.bfloat16
NPBF16 = ml_dtypes.bfloat16

# Problem constants
B, S, D, O = 2, 2048, 4096, 4096
E, R = 8, 16
ER = E * R  # 128
SCALING = 32.0 / 16.0

# Sharding: 8 token groups, W replicated
N_CORES = 8
TG = 8
T = (B * S) // TG       # 512 tokens per core
KT = D // 128           # 32 contraction tiles
OTN = O // 128          # 32 out tiles per core
XC = 8                  # x DMA chunks
KPC = KT // XC          # 4 k-tiles per chunk
NWARM = 8              # PE warm-up matmuls


def build_body(nc, tc, tensors):
    xT, wT, aT, gT, bT, bias2, Rm, out = tensors
    OP = mybir.AluOpType
    ACT = mybir.ActivationFunctionType

    with (
        tc.tile_pool(name="xp", bufs=XC) as xp,
        tc.tile_pool(name="wp", bufs=8) as wp,
        tc.tile_pool(name="cst", bufs=1) as cst,
        tc.tile_pool(name="gw", bufs=1) as gw,
        tc.tile_pool(name="outp", bufs=4) as outp,
        tc.tile_pool(name="ps", bufs=8, space="PSUM") as ps,
    ):
        # ---- DMA program.  The gate/lora-A/x slices for each group of 4
        #      k-tiles are fused host-side into one contiguous per-partition
        #      payload (~5KB lines, no small packets); chunks alternate
        #      across the sync and scalar rings.  W queues strictly behind
        #      the chunks on the sync ring; bias/Rm/bT slot in between the
        #      first W tiles (needed only from ~40us). ----
        PK = E + ER + T          # one k-tile's payload (g | a | x)
        CW = KPC * PK
        w_tiles = [wp.tile([128, KT * 128], BF16, tag="w", name=f"w{ot}")
                   for ot in range(OTN)]

        # even chunks + the first four W tiles interleaved on the (faster)
        # sync ring, odd chunks on the scalar ring; W0..W3 feed the
        # staggered prefix k-loops that fill the PE during the x stream.
        x_tiles = []
        for c in range(XC):
            eng = nc.sync if c % 2 == 0 else nc.scalar
            xc_t = xp.tile([128, CW], BF16, tag="x", name=f"x{c}")
            eng.dma_start(out=xc_t[:], in_=xT[:, c, :])
            x_tiles.append(xc_t)
            if c % 2 == 0:
                j = c // 2
                nc.sync.dma_start(out=w_tiles[j][:], in_=wT[:, j, :])

        # the scheduler would otherwise hoist these ahead of the x chunks
        # (stealing completion sems from them); hold them past the x window
        with tc.tile_wait_until(0.022):
            bias_sb = cst.tile([128, OTN], F32)
            nc.sync.dma_start(out=bias_sb[:], in_=bias2[:])
            Rm_sb = cst.tile([E, ER], BF16)
            nc.sync.dma_start(out=Rm_sb[:], in_=Rm[:])
            bT_sb = cst.tile([ER, O], BF16)
            nc.sync.dma_start(out=bT_sb[:], in_=bT[:])

        with tc.tile_wait_until(0.024):
            for ot in range(4, OTN):
                nc.sync.dma_start(out=w_tiles[ot][:], in_=wT[:, ot, :])

        def gs(k):
            """gate_w.T slice [128, E] for k-tile k."""
            o = (k % KPC) * PK
            return x_tiles[k // KPC][:, o:o + E]

        def as_(k):
            """lora_A.T slice [128, ER] for k-tile k."""
            o = (k % KPC) * PK + E
            return x_tiles[k // KPC][:, o:o + ER]

        def xs(k):
            """x.T slice [128, T] for k-tile k."""
            o = (k % KPC) * PK + E + ER
            return x_tiles[k // KPC][:, o:o + T]

        # ---- PE warm-up: dummy matmuls on zeros so the HAM clock gate is
        #      already at 8/8 when the first x chunk lands.  They write the
        #      gate PSUM bank; the real k=0 matmul's start=True wipes them. ----
        warm = cst.tile([128, T], BF16)
        nc.vector.memset(warm[:], 0.0)

        gate_ps = ps.tile([E, T], F32, tag="ps", name="gateps")
        low_ps = ps.tile([ER, T], F32, tag="ps", name="lowps")
        for i in range(NWARM):
            nc.tensor.matmul(gate_ps[:], lhsT=warm[:, :E], rhs=warm[:],
                             start=True, stop=True, skip_group_check=True)

        # ---- phase B group helper (one chunk of an out-tile's k-loop) ----
        pbs = [None] * OTN

        def w_chunk(ot, cc):
            for k in range(cc * KPC, (cc + 1) * KPC):
                nc.tensor.matmul(pbs[ot][:],
                                 lhsT=w_tiles[ot][:, k * 128:(k + 1) * 128],
                                 rhs=xs(k), start=(k == 0), stop=False)

        # ---- phase A (gate/low k-loops) streamed against chunk arrival,
        #      with W0..W3's k-loops staggered in to fill the PE while the
        #      remaining x chunks are still in flight ----
        for j in range(4):
            pbs[j] = ps.tile([128, T], F32, tag="ps", name=f"pb{j}")
        for c in range(XC):
            for k in range(c * KPC, (c + 1) * KPC):
                nc.tensor.matmul(gate_ps[:], lhsT=gs(k),
                                 rhs=xs(k), start=(k == 0), stop=(k == KT - 1),
                                 skip_group_check=(k == 0))
                nc.tensor.matmul(low_ps[:], lhsT=as_(k),
                                 rhs=xs(k), start=(k == 0), stop=(k == KT - 1))
            for j in range(4):
                cc = c - 2 - j
                if cc >= 0:
                    w_chunk(j, cc)
        for j in range(4):
            for cc in range(XC - 2 - j, XC):
                w_chunk(j, cc)

        # ---- gating math in [E, t] layout (DVE/ACT/GPSIMD; overlaps the
        #      first base-W matmul groups on the PE) ----
        # w_e = [l_e >= m2] * sigmoid(2*l_e - m1 - m2) * SCALING
        lowT_sb = cst.tile([ER, T], BF16, tag="lowT")
        g_sb = gw.tile([E, T], F32, tag="g")
        nc.scalar.copy(g_sb[:], gate_ps[:])
        m1 = gw.tile([E, T], F32, tag="m1")
        nc.gpsimd.partition_all_reduce(m1[:], g_sb[:], channels=E,
                                       reduce_op=bass_isa.ReduceOp.max)
        eq = gw.tile([E, T], F32, tag="eq")
        nc.vector.tensor_tensor(eq[:], g_sb[:], m1[:], op=OP.is_equal)
        gm = gw.tile([E, T], F32, tag="gm")
        nc.vector.scalar_tensor_tensor(gm[:], in0=eq[:], scalar=-1e30,
                                       in1=g_sb[:], op0=OP.mult, op1=OP.add)
        m2 = gw.tile([E, T], F32, tag="m2")
        nc.gpsimd.partition_all_reduce(m2[:], gm[:], channels=E,
                                       reduce_op=bass_isa.ReduceOp.max)
        t1 = gw.tile([E, T], F32, tag="t1")
        nc.vector.tensor_tensor(t1[:], m1[:], m2[:], op=OP.add)
        s = gw.tile([E, T], F32, tag="s")
        nc.vector.scalar_tensor_tensor(s[:], in0=g_sb[:], scalar=2.0,
                                       in1=t1[:], op0=OP.mult, op1=OP.subtract)
        sig = gw.tile([E, T], F32, tag="sig")
        nc.scalar.activation(sig[:], s[:], ACT.Sigmoid)
        mask = gw.tile([E, T], F32, tag="mask")
        nc.vector.tensor_tensor(mask[:], g_sb[:], m2[:], op=OP.is_ge)
        wsc = gw.tile([E, T], BF16, tag="wsc")
        nc.vector.scalar_tensor_tensor(wsc[:], in0=sig[:], scalar=SCALING,
                                       in1=mask[:], op0=OP.mult, op1=OP.mult)

        # ---- phase B: out.T[ot] = sum_k W[ot,k]^T @ x.T (+ B^T @ low_w.T) ----
        def stop_group(ot):
            nc.tensor.matmul(pbs[ot][:], lhsT=bT_sb[:, ot * 128:(ot + 1) * 128],
                             rhs=lowT_sb[:], start=False, stop=True)
            o_sb = outp.tile([128, T], BF16, tag="o", name=f"o{ot}")
            nc.vector.tensor_scalar(o_sb[:], pbs[ot][:],
                                    scalar1=bias_sb[:, ot:ot + 1],
                                    scalar2=None, op0=OP.add)
            # last few outputs go out on the otherwise-idle ACT ring so the
            # final drain doesn't serialize behind one DMA queue
            eng = nc.scalar if ot >= OTN - 3 else nc.gpsimd
            eng.dma_start(out=out[:, ot, :], in_=o_sb[:])

        for ot in range(4, OTN):
            pbs[ot] = ps.tile([128, T], F32, tag="ps", name=f"pb{ot}")
            if ot == OTN - 1:
                # close out the previous group before the final W k-loop so
                # only one stop+store trails the last matmul
                stop_group(OTN - 2)
            for cc in range(XC):
                w_chunk(ot, cc)
            if ot == 4:
                # replicate each expert weight over its 16 ranks via a tiny
                # matmul, then fold into the rank-space activations.  Emitted
                # after ot4's W MMs so the PE never waits on the gating chain.
                wrep_ps = ps.tile([ER, T], F32, tag="ps", name="wrep")
                nc.tensor.matmul(wrep_ps[:], lhsT=Rm_sb[:], rhs=wsc[:],
                                 start=True, stop=True)
                wrep_sb = gw.tile([ER, T], F32, tag="wrepsb")
                nc.scalar.copy(wrep_sb[:], wrep_ps[:])
                # low_w.T = low.T * w_rep (DVE: one PSUM operand only)
                nc.vector.tensor_tensor(lowT_sb[:], low_ps[:], wrep_sb[:],
                                        op=OP.mult)
            if ot == 5:
                for j in range(5):
                    stop_group(j)
            if ot >= 6 and ot < OTN - 1:
                stop_group(ot - 1)
        stop_group(OTN - 1)


def build_module(debug=False):
    nc = bacc.Bacc("TRN2", target_bir_lowering=False, debug=debug)
    CW = KPC * (E + ER + T)
    xT = nc.dram_tensor("xT", [128, XC, CW], BF16, kind="ExternalInput")
    wT = nc.dram_tensor("wT", [128, OTN, KT * 128], BF16, kind="ExternalInput")
    aT = None
    gT = None
    bT = nc.dram_tensor("bT", [ER, O], BF16, kind="ExternalInput")
    bias2 = nc.dram_tensor("bias2", [128, OTN], F32, kind="ExternalInput")
    Rm = nc.dram_tensor("Rm", [E, ER], BF16, kind="ExternalInput")
    out = nc.dram_tensor("out", [128, OTN, T], BF16, kind="ExternalOutput")
    with tile.TileContext(nc) as tc:
        build_body(nc, tc, (xT, wT, aT, gT, bT, bias2, Rm, out))
    nc.compile()
    return nc


def shard_inputs(x, gate_w, base_w, base_b, lora_A, lora_B):
    """FULL inputs -> list of 8 per-core input maps (host-side, free)."""
    x = np.asarray(x, dtype=np.float32)
    gate_w = np.asarray(gate_w, dtype=np.float32)
    base_w = np.asarray(base_w, dtype=np.float32)
    base_b = np.asarray(base_b, dtype=np.float32)
    lora_A = np.asarray(lora_A, dtype=np.float32)
    lora_B = np.asarray(lora_B, dtype=np.float32)

    xf = x.reshape(B * S, D)
    # replicated tensors; gate/lora-A slices fused into each x chunk payload
    gp = gate_w.T.reshape(XC, KPC, 128, E).transpose(2, 0, 1, 3)  # [128,XC,KPC,E]
    A_flat = lora_A.reshape(ER, D)
    ap = A_flat.T.reshape(XC, KPC, 128, ER).transpose(2, 0, 1, 3)
    B_flat = lora_B.transpose(0, 2, 1).reshape(ER, O)   # [er, o]
    bT = np.ascontiguousarray(B_flat).astype(NPBF16)
    Rm = np.repeat(np.eye(E, dtype=np.float32), R, axis=1).astype(NPBF16)
    wT = np.ascontiguousarray(
        base_w.reshape(OTN, 128, KT, 128).transpose(3, 0, 2, 1)
        .reshape(128, OTN, KT * 128)).astype(NPBF16)
    bias2 = np.ascontiguousarray(base_b.reshape(OTN, 128).T)
    in_maps = []
    for c in range(N_CORES):
        x_c = xf[c * T:(c + 1) * T]                         # [T, D]
        xp_ = x_c.T.reshape(XC, KPC, 128, T).transpose(2, 0, 1, 3)
        # per-k payload blocks [g_k | a_k | x_k], flattened per chunk
        xT = np.ascontiguousarray(np.concatenate(
            [gp, ap, xp_], axis=3).reshape(128, XC, KPC * (E + ER + T))
            ).astype(NPBF16)
        in_maps.append({"xT": xT, "wT": wT,
                        "bT": bT, "bias2": bias2, "Rm": Rm})
    return in_maps


def gather_outputs(results):
    """list of 8 per-core result maps -> FULL output [B, S, O]."""
    full = np.empty((B * S, O), dtype=np.float32)
    for c in range(N_CORES):
        oc = np.asarray(results[c]["out"], dtype=np.float32)  # [128, OTN, T]
        full[c * T:(c + 1) * T, :] = oc.transpose(2, 1, 0).reshape(T, O)
    return full.reshape(B, S, O)


_NC_CACHE = {}


def _get_module():
    if "nc" not in _NC_CACHE:
        _NC_CACHE["nc"] = build_module()
    return _NC_CACHE["nc"]


def run_sharded(in_maps, **run_kwargs):
    nc = _get_module()
    return run_bass_kernel_spmd(nc, in_maps, list(range(N_CORES)), **run_kwargs)


def kernel(x, gate_w, base_w, base_b, lora_A, lora_B):
    in_maps = shard_inputs(x, gate_w, base_w, base_b, lora_A, lora_B)
    res = run_sharded(in_maps)
    return gather_outputs(res.results)


# revision 19
# speedup vs baseline: 1.0197x; 1.0013x over previous
"""Trainium2 Bass kernel for a LoRA-MoE layer (gate top-2 softmax routing +
dense base linear + per-expert low-rank adapters), SPMD across 8 NeuronCores.

Math (per token t):
    logits = x @ gate_w.T                      # [E]
    top-2 softmax over logits -> dense w[E] (0 for non-selected)
    out = x @ base_w.T + base_b
        + SCALING * sum_e w[e] * (x @ lora_A[e].T) @ lora_B[e].T

Key identities:
  * w folded into rank-space activations: lora_out = (low * w_rep) @ B_all.T
    with low = x @ A_all.T (A_all: [E*R, D]) -> whole MoE-LoRA is two dense
    matmuls + tiny gating vector math.
  * top-2 softmax via sigmoid: w_e = [l_e >= m2] * sigmoid(2*l_e - m1 - m2)
    (for the top-1 expert this is sigmoid(l1-l2), for top-2 sigmoid(l2-l1)).

Sharding: 8-way data parallel over tokens (T=512 tokens per core), base W
replicated and streamed.  This halves the x-load + phase-A serial head vs a
token x out-feature split; W streaming needs only ~150 GB/s per core.

Performance structure (~269us vs 360us f32r baseline; PE matmul-issue
bound at ~220ns per 512-col matmul from ~30us onward):
  * all matmul operands bf16 (host cast, free) -> PE rate unchanged, HBM
    bytes halved.  Output returned as bf16 and upcast on host.
  * gate/lora-A/x slices for each group of 4 k-tiles are fused host-side
    into one contiguous per-partition DMA payload (no small packets);
    chunks alternate across the sync/scalar HW-DGE rings, W0..W3
    interleave behind the early chunks, everything else is held past the
    x window (the DMA completion-sem pool only allows ~9 outstanding
    transfers; extra triggers would steal sems and stall the chunks).
  * 8 dummy matmuls on zeros at t~8us hold the PE HAM clock-gate at 8/8
    until the first x chunk lands.
  * phase A (gate/low) streams against chunk arrival with W0..W3's k-loops
    staggered in so the PE never idles during the x stream.
  * single shared 8-slot PSUM pool; the gating vector chain
    (DVE/ACT/GPSIMD) hides behind W matmul groups; per-out-tile B-adapter
    "stop" matmuls trail one group, and the second-to-last stop is hoisted
    before the final W group to shorten the tail.
"""

import numpy as np
import ml_dtypes

import concourse.bass as bass
import concourse.bass_isa as bass_isa
import concourse.mybir as mybir
import concourse.tile as tile
from concourse import bacc
from concourse.bass_utils import run_bass_kernel_spmd

F32 = mybir.dt.float32
BF16 = mybir.dt.bfloat16
NPBF16 = ml_dtypes.bfloat16

# Problem constants
B, S, D, O = 2, 2048, 4096, 4096
E, R = 8, 16
ER = E * R  # 128
SCALING = 32.0 / 16.0

# Sharding: 8 token groups, W replicated
N_CORES = 8
TG = 8
T = (B * S) // TG       # 512 tokens per core
KT = D // 128           # 32 contraction tiles
OTN = O // 128          # 32 out tiles per core
XC = 8                  # x DMA chunks
KPC = KT // XC          # 4 k-tiles per chunk
NWARM = 8              # PE warm-up matmuls


def build_body(nc, tc, tensors):
    xT, wT, aT, gT, bT, bias2, Rm, out = tensors
    OP = mybir.AluOpType
    ACT = mybir.ActivationFunctionType

    with (
        tc.tile_pool(name="xp", bufs=XC) as xp,
        tc.tile_pool(name="wp", bufs=8) as wp,
        tc.tile_pool(name="cst", bufs=1) as cst,
        tc.tile_pool(name="gw", bufs=1) as gw,
        tc.tile_pool(name="outp", bufs=4) as outp,
        tc.tile_pool(name="ps", bufs=8, space="PSUM") as ps,
    ):
        # ---- DMA program.  The gate/lora-A/x slices for each group of 4
        #      k-tiles are fused host-side into one contiguous per-partition
        #      payload (~5KB lines, no small packets); chunks alternate
        #      across the sync and scalar rings.  W queues strictly behind
        #      the chunks on the sync ring; bias/Rm/bT slot in between the
        #      first W tiles (needed only from ~40us). ----
        PK = E + ER + T          # one k-tile's payload (g | a | x)
        CW = KPC * PK
        w_tiles = [wp.tile([128, KT * 128], BF16, tag="w", name=f"w{ot}")
                   for ot in range(OTN)]

        # even chunks + the first four W tiles interleaved on the (faster)
        # sync ring, odd chunks on the scalar ring; W0..W3 feed the
        # staggered prefix k-loops that fill the PE during the x stream.
        x_tiles = []
        for c in range(XC):
            eng = nc.sync if c % 2 == 0 else nc.scalar
            xc_t = xp.tile([128, CW], BF16, tag="x", name=f"x{c}")
            eng.dma_start(out=xc_t[:], in_=xT[:, c, :])
            x_tiles.append(xc_t)
            if c % 2 == 0:
                j = c // 2
                nc.sync.dma_start(out=w_tiles[j][:], in_=wT[:, j, :])

        # the scheduler would otherwise hoist these ahead of the x chunks
        # (stealing completion sems from them); hold them past the x window
        with tc.tile_wait_until(0.022):
            bias_sb = cst.tile([128, OTN], F32)
            nc.sync.dma_start(out=bias_sb[:], in_=bias2[:])
            Rm_sb = cst.tile([E, ER], BF16)
            nc.sync.dma_start(out=Rm_sb[:], in_=Rm[:])
            bT_sb = cst.tile([ER, O], BF16)
            nc.sync.dma_start(out=bT_sb[:], in_=bT[:])

        with tc.tile_wait_until(0.024):
            for ot in range(4, OTN):
                nc.sync.dma_start(out=w_tiles[ot][:], in_=wT[:, ot, :])

        def gs(k):
            """gate_w.T slice [128, E] for k-tile k."""
            o = (k % KPC) * PK
            return x_tiles[k // KPC][:, o:o + E]

        def as_(k):
            """lora_A.T slice [128, ER] for k-tile k."""
            o = (k % KPC) * PK + E
            return x_tiles[k // KPC][:, o:o + ER]

        def xs(k):
            """x.T slice [128, T] for k-tile k."""
            o = (k % KPC) * PK + E + ER
            return x_tiles[k // KPC][:, o:o + T]

        # ---- PE warm-up: dummy matmuls on zeros so the HAM clock gate is
        #      already at 8/8 when the first x chunk lands.  They write the
        #      gate PSUM bank; the real k=0 matmul's start=True wipes them. ----
        warm = cst.tile([128, T], BF16)
        nc.vector.memset(warm[:], 0.0)

        gate_ps = ps.tile([E, T], F32, tag="ps", name="gateps")
        low_ps = ps.tile([ER, T], F32, tag="ps", name="lowps")
        for i in range(NWARM):
            nc.tensor.matmul(gate_ps[:], lhsT=warm[:, :E], rhs=warm[:],
                             start=True, stop=True, skip_group_check=True)

        # ---- phase B group helper (one chunk of an out-tile's k-loop) ----
        pbs = [None] * OTN

        def w_chunk(ot, cc):
            for k in range(cc * KPC, (cc + 1) * KPC):
                nc.tensor.matmul(pbs[ot][:],
                                 lhsT=w_tiles[ot][:, k * 128:(k + 1) * 128],
                                 rhs=xs(k), start=(k == 0), stop=False)

        # ---- phase A (gate/low k-loops) streamed against chunk arrival,
        #      with W0..W3's k-loops staggered in to fill the PE while the
        #      remaining x chunks are still in flight ----
        for j in range(4):
            pbs[j] = ps.tile([128, T], F32, tag="ps", name=f"pb{j}")
        for c in range(XC):
            for k in range(c * KPC, (c + 1) * KPC):
                nc.tensor.matmul(gate_ps[:], lhsT=gs(k),
                                 rhs=xs(k), start=(k == 0), stop=(k == KT - 1),
                                 skip_group_check=(k == 0))
                nc.tensor.matmul(low_ps[:], lhsT=as_(k),
                                 rhs=xs(k), start=(k == 0), stop=(k == KT - 1))
            for j in range(4):
                cc = c - 2 - j
                if cc >= 0:
                    w_chunk(j, cc)
        for j in range(4):
            for cc in range(XC - 2 - j, XC):
                w_chunk(j, cc)

        # ---- gating math in [E, t] layout (DVE/ACT/GPSIMD; overlaps the
        #      first base-W matmul groups on the PE) ----
        # w_e = [l_e >= m2] * sigmoid(2*l_e - m1 - m2) * SCALING
        lowT_sb = cst.tile([ER, T], BF16, tag="lowT")
        g_sb = gw.tile([E, T], F32, tag="g")
        nc.scalar.copy(g_sb[:], gate_ps[:])
        m1 = gw.tile([E, T], F32, tag="m1")
        nc.gpsimd.partition_all_reduce(m1[:], g_sb[:], channels=E,
                                       reduce_op=bass_isa.ReduceOp.max)
        eq = gw.tile([E, T], F32, tag="eq")
        nc.vector.tensor_tensor(eq[:], g_sb[:], m1[:], op=OP.is_equal)
        gm = gw.tile([E, T], F32, tag="gm")
        nc.vector.scalar_tensor_tensor(gm[:], in0=eq[:], scalar=-1e30,
                                       in1=g_sb[:], op0=OP.mult, op1=OP.add)
        m2 = gw.tile([E, T], F32, tag="m2")
        nc.gpsimd.partition_all_reduce(m2[:], gm[:], channels=E,
                                       reduce_op=bass_isa.ReduceOp.max)
        t1 = gw.tile([E, T], F32, tag="t1")
        nc.vector.tensor_tensor(t1[:], m1[:], m2[:], op=OP.add)
        s = gw.tile([E, T], F32, tag="s")
        nc.vector.scalar_tensor_tensor(s[:], in0=g_sb[:], scalar=2.0,
                                       in1=t1[:], op0=OP.mult, op1=OP.subtract)
        sig = gw.tile([E, T], F32, tag="sig")
        nc.scalar.activation(sig[:], s[:], ACT.Sigmoid)
        mask = gw.tile([E, T], F32, tag="mask")
        nc.vector.tensor_tensor(mask[:], g_sb[:], m2[:], op=OP.is_ge)
        wsc = gw.tile([E, T], BF16, tag="wsc")
        nc.vector.scalar_tensor_tensor(wsc[:], in0=sig[:], scalar=SCALING,
                                       in1=mask[:], op0=OP.mult, op1=OP.mult)

        # ---- phase B: out.T[ot] = sum_k W[ot,k]^T @ x.T (+ B^T @ low_w.T) ----
        def stop_group(ot):
            nc.tensor.matmul(pbs[ot][:], lhsT=bT_sb[:, ot * 128:(ot + 1) * 128],
                             rhs=lowT_sb[:], start=False, stop=True)
            o_sb = outp.tile([128, T], BF16, tag="o", name=f"o{ot}")
            nc.vector.tensor_scalar(o_sb[:], pbs[ot][:],
                                    scalar1=bias_sb[:, ot:ot + 1],
                                    scalar2=None, op0=OP.add)
            # last few outputs go out on the otherwise-idle ACT ring so the
            # final drain doesn't serialize behind one DMA queue
            eng = nc.scalar if ot >= OTN - 3 else nc.gpsimd
            eng.dma_start(out=out[:, ot, :], in_=o_sb[:])

        for ot in range(4, OTN):
            pbs[ot] = ps.tile([128, T], F32, tag="ps", name=f"pb{ot}")
            if ot == OTN - 1:
                # close out the previous group before the final W k-loop so
                # only one stop+store trails the last matmul
                stop_group(OTN - 2)
            for cc in range(XC):
                w_chunk(ot, cc)
            if ot == 4:
                # replicate each expert weight over its 16 ranks via a tiny
                # matmul, then fold into the rank-space activations.  Emitted
                # after ot4's W MMs so the PE never waits on the gating chain.
                wrep_ps = ps.tile([ER, T], F32, tag="ps", name="wrep")
                nc.tensor.matmul(wrep_ps[:], lhsT=Rm_sb[:], rhs=wsc[:],
                                 start=True, stop=True)
                wrep_sb = gw.tile([ER, T], F32, tag="wrepsb")
                nc.scalar.copy(wrep_sb[:], wrep_ps[:])
                # low_w.T = low.T * w_rep (DVE: one PSUM operand only)
                nc.vector.tensor_tensor(lowT_sb[:], low_ps[:], wrep_sb[:],
                                        op=OP.mult)
            if ot == 5:
                for j in range(5):
                    stop_group(j)
            if ot >= 6 and ot < OTN - 1:
                stop_group(ot - 1)
        stop_group(OTN - 1)


def build_module(debug=False):
    nc = bacc.Bacc("TRN2", target_bir_lowering=False, debug=debug)
    CW = KPC * (E + ER + T)
    xT = nc.dram_tensor("xT", [128, XC, CW], BF16, kind="ExternalInput")
    wT = nc.dram_tensor("wT", [128, OTN, KT * 128], BF16, kind="ExternalInput")
    aT = None
    gT = None
    bT = nc.dram_tensor("bT", [ER, O], BF16, kind="ExternalInput")
    bias2 = nc.dram_tensor("bias2", [128, OTN], F32, kind="ExternalInput")
    Rm = nc.dram_tensor("Rm", [E, ER], BF16, kind="ExternalInput")
    out = nc.dram_tensor("out", [128, OTN, T], BF16, kind="ExternalOutput")
    with tile.TileContext(nc) as tc:
        build_body(nc, tc, (xT, wT, aT, gT, bT, bias2, Rm, out))
    nc.compile()
    return nc


def shard_inputs(x, gate_w, base_w, base_b, lora_A, lora_B):
    """FULL inputs -> list of 8 per-core input maps (host-side, free)."""
    x = np.asarray(x, dtype=np.float32)
    gate_w = np.asarray(gate_w, dtype=np.float32)
    base_w = np.asarray(base_w, dtype=np.float32)
    base_b = np.asarray(base_b, dtype=np.float32)
    lora_A = np.asarray(lora_A, dtype=np.float32)
    lora_B = np.asarray(lora_B, dtype=np.float32)

    xf = x.reshape(B * S, D)
    # replicated tensors; gate/lora-A slices fused into each x chunk payload
    gp = gate_w.T.reshape(XC, KPC, 128, E).transpose(2, 0, 1, 3)  # [128,XC,KPC,E]
    A_flat = lora_A.reshape(ER, D)
    ap = A_flat.T.reshape(XC, KPC, 128, ER).transpose(2, 0, 1, 3)
    B_flat = lora_B.transpose(0, 2, 1).reshape(ER, O)   # [er, o]
    bT = np.ascontiguousarray(B_flat).astype(NPBF16)
    Rm = np.repeat(np.eye(E, dtype=np.float32), R, axis=1).astype(NPBF16)
    wT = np.ascontiguousarray(
        base_w.reshape(OTN, 128, KT, 128).transpose(3, 0, 2, 1)
        .reshape(128, OTN, KT * 128)).astype(NPBF16)
    bias2 = np.ascontiguousarray(base_b.reshape(OTN, 128).T)
    in_maps = []
    for c in range(N_CORES):
        x_c = xf[c * T:(c + 1) * T]                         # [T, D]
        xp_ = x_c.T.reshape(XC, KPC, 128, T).transpose(2, 0, 1, 3)
        # per-k payload blocks [g_k | a_k | x_k], flattened per chunk
        xT = np.ascontiguousarray(np.concatenate(
            [gp, ap, xp_], axis=3).reshape(128, XC, KPC * (E + ER + T))
            ).astype(NPBF16)
        in_maps.append({"xT": xT, "wT": wT,
                        "bT": bT, "bias2": bias2, "Rm": Rm})
    return in_maps


def gather_outputs(results):
    """list of 8 per-core result maps -> FULL output [B, S, O]."""
    full = np.empty((B * S, O), dtype=np.float32)
    for c in range(N_CORES):
        oc = np.asarray(results[c]["out"], dtype=np.float32)  # [128, OTN, T]
        full[c * T:(c + 1) * T, :] = oc.transpose(2, 1, 0).reshape(T, O)
    return full.reshape(B, S, O)


_NC_CACHE = {}


def _get_module():
    if "nc" not in _NC_CACHE:
        _NC_CACHE["nc"] = build_module()
    return _NC_CACHE["nc"]


def run_sharded(in_maps, **run_kwargs):
    nc = _get_module()
    return run_bass_kernel_spmd(nc, in_maps, list(range(N_CORES)), **run_kwargs)


def kernel(x, gate_w, base_w, base_b, lora_A, lora_B):
    in_maps = shard_inputs(x, gate_w, base_w, base_b, lora_A, lora_B)
    res = run_sharded(in_maps)
    return gather_outputs(res.results)
